# revision 1
# baseline (speedup 1.0000x reference)
"""Trainium2 Bass kernel for nn_DenseCapsuleLayer.

Reference computation:
    u_hat[b, j, k, n] = sum_m W[0, j, idx[b,k], n, m] * x[b, idx[b,k], m]
with idx[b, :] the ascending indices of the NZC=1152 non-zero child capsules
of batch b (x is zero elsewhere).

Strategy (8 NeuronCores, 2-way parent-capsule x 4-way batch mesh):
  * Core c owns j in [16*(c%2), 16*(c%2)+16) and b in [8*(c//2), 8*(c//2)+8).
  * Each core computes the DENSE map u_full[b, i, jl, n] for ALL i (x is zero
    at non-selected i, so u_full there is zero and is discarded); the
    select/compaction gather over i and the unshard/concat happen on the
    host.
  * Per 16-wide child-capsule chunk g (i = 16g+il), the PE computes
        out[(il,bl), (jl,n)] = sum_m x[b, 16g+il, m] * W[j, 16g+il, n, m]
    as ONE K=128 matmul: the stationary operand is a [128,128]
    block-diagonal packing of the core's x slice (8 batches) built ON DEVICE
    by a broadcast multiply with a static 0/1 mask (x ships compact, 8
    floats per row), the moving operand is the core's W slice pre-transposed
    to [i, m, (jl,n)] (256 free columns).  All matmuls keep base partition
    0 (mixing stationary base partitions crashes this device).

Toolchain constraints: every lowered instruction accepts ONE sync-wait
command and Tile emits a wait per dependency, so dummy ops absorb all but
one dependency per real instruction, SP nops "park" the kernel-tail drain's
wait list, and a BIR post-pass drops DMA waits that are provably implied by
the single wait that is kept.
"""

import numpy as np

B, I, J, M, N = 32, 2304, 32, 8, 16
NZC = I // 2
NCORES = 8
JL = J // 2               # parent capsules per core (16)
JN = JL * N               # 256
BL = B // 4               # batches per core (8)
NCHUNK = I // 16          # 144 chunks of 16 child capsules
NSTRIP = 8
CH_PER_STRIP = NCHUNK // NSTRIP  # 18
PAD = 4                   # o_sb pad elements (see dummy B)

_CACHE = {}


def _build_program():
    import concourse.bass as bass
    import concourse.mybir as mybir
    import concourse.tile as tile

    f32 = mybir.dt.float32
    nc = bass.Bass()

    # wb[g, (il,m), 0:256]   = W[j, 16g+il, n, m]  (moving operand)
    # wb[g, (il,m), 256:264] = x[b, 16g+il, m] for the core's 8 batches
    wb = nc.declare_dram_parameter("wb", [NCHUNK, 128, JN + BL], f32,
                                   isOutput=False)
    # msk[(il,m), (il',bl)] = 1.0 iff il == il'
    msk = nc.declare_dram_parameter("msk", [128, 128], f32, isOutput=False)
    u = nc.declare_dram_parameter(
        "u", [128, NSTRIP * (CH_PER_STRIP * JN + PAD)], f32, isOutput=True
    )
    SJN = CH_PER_STRIP * JN + PAD

    with tile.TileContext(nc, pool_alloc_mode="queue") as tc:
        with (
            tc.tile_pool(name="wpool", bufs=3) as wpool,
            tc.tile_pool(name="bdpool", bufs=3) as bdpool,
            tc.tile_pool(name="opool", bufs=3) as opool,
            tc.tile_pool(name="ppool", bufs=6, space="PSUM") as ppool,
            tc.tile_pool(name="dpool", bufs=1, space="PSUM") as dpool,
            tc.tile_pool(name="zpool", bufs=1) as zpool,
        ):
            # dmy: PE dummy-matmul targets (each column written exactly once)
            # sig: written by DVE right after each PSUM->SBUF copy; read by
            #      DVE dummy A to advance the DVE clock across strips
            dmy = dpool.tile([1, 160], f32, tag="d")
            sig = zpool.tile([32, 160], f32, tag="sig")
            sig2 = zpool.tile([1, 128], f32, tag="sig2")
            z_sb = zpool.tile([128, PAD], f32, tag="z")
            mask_t = zpool.tile([128, 128], f32, tag="msk")
            nc.vector.memset(z_sb[:, :], 0.0)
            d_msk = nc.sync.dma_start(out=mask_t[:, :], in_=msk[:, :])
            # absorbs the mask-load wait on the DVE queue
            nc.vector.tensor_copy(sig2[0:1, 120:121], mask_t[0:1, 0:1])
            mask3 = mask_t.rearrange("p (r c) -> p r c", r=16)

            all_dmas = [d_msk]
            cps = []

            def park_wait(dep, prev=None):
                w = nc.sync.nop(nofuse=True, hint="park")
                tile.add_dep_helper(w.ins, dep.ins, sync=True, reason="park")
                if prev is not None:
                    tile.add_dep_helper(w.ins, prev.ins, sync=False, reason="ord")
                return w

            for s in range(NSTRIP):
                glo = s * CH_PER_STRIP
                w_sb = wpool.tile([128, CH_PER_STRIP, JN + BL], f32, tag="w")
                # carries (s>=3) the w-slot WAR: PE readers of strip s-3 (the
                # DVE readers and the slot WAW are implied; post-pass below)
                d_in = nc.sync.dma_start(
                    out=w_sb[:, :, :],
                    in_=wb[glo : glo + CH_PER_STRIP].rearrange("g p c -> p g c"),
                )
                all_dmas.append(d_in)
                # absorb the strip-DMA wait on the PE queue...
                sdum = nc.tensor.matmul(
                    dmy[0:1, s : s + 1],
                    w_sb[0:32, 0, 0:1],
                    w_sb[0:32, 0, 0:1],
                    start=True,
                    stop=True,
                )
                # ...and on the DVE queue (for the bd builder muls)
                sdumv = nc.vector.tensor_copy(
                    sig2[0:1, 8 + s : 9 + s], w_sb[0:1, 0, JN : JN + 1]
                )
                o_sb = opool.tile([128, CH_PER_STRIP * JN + PAD], f32, tag="o")
                bdt = bdpool.tile([128, CH_PER_STRIP, 128], f32, tag="bd")
                adum = None
                if s >= 3:
                    # dummy A: advances the DVE clock past all of strip s-3's
                    # copies (covers copy/bd-mul WAWs and dummy B's pad WAW)
                    adum = nc.vector.tensor_copy(
                        sig2[0:1, 96 + s : 97 + s], sig[0:1, s - 3 : s - 2]
                    )
                # dummy B: pad write carries the o_sb slot-reuse WAR (the
                # out-DMA of strip s-3 read the pad too, so the WAR re-forms)
                bdum = nc.vector.tensor_copy(
                    o_sb[:, CH_PER_STRIP * JN : CH_PER_STRIP * JN + PAD],
                    z_sb[:, :],
                )
                if adum is not None:
                    tile.add_dep_helper(
                        bdum.ins, adum.ins, sync=False, reason="A before B"
                    )
                for gl in range(CH_PER_STRIP):
                    gg = s * CH_PER_STRIP + gl  # global chunk index
                    # build the block-diagonal stationary on device:
                    # bdt[p, (il', bl)] = x[p-row] * mask[p, (il', bl)]
                    mul = nc.vector.tensor_mul(
                        bdt[:, gl, :].rearrange("p (r c) -> p r c", r=16),
                        w_sb[:, gl : gl + 1, JN : JN + BL].broadcast_to(
                            [128, 16, BL]
                        ),
                        mask3,
                    )
                    pair = gg // 2
                    if pair >= 6:
                        # the bank-WAR coverage via gdum's mul-tick needs this
                        # mul scheduled AFTER the copy that frees the pair's
                        # PSUM bank (6 pairs back) on the DVE queue
                        tile.add_dep_helper(
                            mul.ins, cps[pair - 6].ins, sync=False,
                            reason="mul after bank-freeing copy",
                        )
                    if gl == 0:
                        tile.add_dep_helper(
                            mul.ins, sdumv.ins, sync=False,
                            reason="dve strip dummy before muls",
                        )
                        if adum is not None:
                            tile.add_dep_helper(
                                mul.ins, adum.ins, sync=False,
                                reason="A before first mul",
                            )
                    if gl % 2 == 0:
                        ps = ppool.tile([128, 2, JN], f32, tag="ps")
                    # absorbs (on PE) the RAW wait on the bd mul, which also
                    # covers the PSUM-bank WAR (the freeing copy ran earlier
                    # on the same DVE queue)
                    gdum = nc.tensor.matmul(
                        dmy[0:1, 8 + gg : 9 + gg],
                        bdt[0:32, gl, 0:1],
                        bdt[0:32, gl, 0:1],
                        start=True,
                        stop=True,
                    )
                    mm = nc.tensor.matmul(
                        ps[:, gl % 2, :],
                        bdt[:, gl, :],
                        w_sb[0:128, gl, 0:JN],
                        start=True,
                        stop=True,
                    )
                    tile.add_dep_helper(
                        mm.ins, gdum.ins, sync=False, reason="gdum before MM"
                    )
                    if gl == 0:
                        tile.add_dep_helper(
                            mm.ins, sdum.ins, sync=False,
                            reason="strip dummy before first MM",
                        )
                    # one copy per chunk PAIR; carries only its RAW wait
                    if gl % 2 == 1:
                        cp = nc.vector.tensor_copy(
                            o_sb[:, (gl - 1) * JN : (gl + 1) * JN],
                            ps.rearrange("p a b -> p (a b)"),
                        )
                        tile.add_dep_helper(
                            cp.ins, bdum.ins, sync=False, reason="B before copies"
                        )
                        cps.append(cp)
                    last_mm = mm
                # sig write (one per strip): RAW on the strip's last copy
                # keeps DVE ordering; read by dummy A two strips later
                last_sigw = nc.vector.tensor_copy(
                    sig[0:32, s : s + 1],
                    o_sb[0:32, (CH_PER_STRIP - 1) * JN : (CH_PER_STRIP - 1) * JN + 1],
                )
                # carries only its DVE wait; lane wait dropped by post-pass.
                # Issued from the ACT sequencer so input (SP) and output
                # DMA streams overlap.
                d_out = nc.scalar.dma_start(
                    out=u[:, s * SJN : (s + 1) * SJN], in_=o_sb[:, :]
                )
                all_dmas.append(d_out)
            # tail parking: cover the last 8 DMAs + engine tails so the
            # kernel-tail drain has at most one wait left
            prev = None
            for d in all_dmas + [last_mm, last_sigw]:
                prev = park_wait(d, prev)

    # Single-wait legalization: keep the strongest wait per DMA (PE if
    # present, else DVE) — the dropped DMAHW/DVE waits are implied by it
    # through the dummy-op ordering chains (the kept tick is only reached
    # after the dropped dependencies completed).
    import concourse.mybir as mybir2

    for blk in nc.m.functions[0].blocks:
        for inst in blk.instructions:
            si = inst.sync_info
            if si is None or not si.on_wait or len(si.on_wait) < 2:
                continue
            if type(inst).__name__ != "InstDMACopy":
                raise RuntimeError(f"unexpected multi-wait {inst.name}")
            pe = [w for w in si.on_wait if w.ant_name.startswith("PE")]
            dve = [w for w in si.on_wait if w.ant_name.startswith("DVE")]
            dma = [w for w in si.on_wait if w.ant_name.startswith("DMAHW")]
            if len(pe) + len(dve) + len(dma) != len(si.on_wait):
                raise RuntimeError(f"unexpected wait mix on {inst.name}")
            keep = pe[:1] or dve[:1]
            if len(keep) != 1:
                raise RuntimeError(f"no engine wait to keep on {inst.name}")
            inst.sync_info = mybir2.SyncInfo(
                on_wait=keep, on_update=list(si.on_update or [])
            )
    return nc


def _get_program():
    if "nc" not in _CACHE:
        _CACHE["nc"] = _build_program()
    return _CACHE["nc"]


def _host_prep(input, W):
    """Build per-core in_maps. input: [B, I, M]; W: [1, J, I, N, M]."""
    x = np.ascontiguousarray(input, dtype=np.float32)
    W0 = np.ascontiguousarray(W[0], dtype=np.float32)  # [J, I, N, M]

    # mask[(il, m), (il', bl)] = 1 iff il == il'
    il_row = (np.arange(128) // M)[:, None]
    il_col = (np.arange(128) // BL)[None, :]
    mask = (il_row == il_col).astype(np.float32)

    wts = []
    for jg in range(2):
        ws = W0[JL * jg : JL * jg + JL]                 # [JL, I, N, M]
        wts.append(ws.transpose(1, 3, 0, 2).reshape(NCHUNK, 128, JN))
    xcs = []
    for bg in range(4):
        xs = x[BL * bg : BL * bg + BL]                  # [BL, I, M]
        # xc[g, (il, m), bl] = x[bl, 16g+il, m]
        xcs.append(xs.transpose(1, 2, 0).reshape(NCHUNK, 128, BL))

    in_maps = []
    for c in range(NCORES):
        jg, bg = c % 2, c // 2
        in_maps.append(
            {"wb": np.concatenate([wts[jg], xcs[bg]], axis=2), "msk": mask}
        )
    return in_maps


def _host_finish(input, results):
    """Gather selected child capsules and unshard over (j, b)."""
    mask = input.sum(axis=2) != 0.0                     # [B, I]
    keyv = np.where(mask, np.arange(I)[None, :], I)
    sidx = np.sort(keyv, axis=1)[:, :NZC]               # [B, NZC]

    ufull = np.empty((B, I, J, N), dtype=np.float32)
    for c in range(NCORES):
        jg, bg = c % 2, c // 2
        uc = results[c]["u"].reshape(128, NSTRIP, CH_PER_STRIP * JN + PAD)
        uc = uc[:, :, : CH_PER_STRIP * JN].reshape(16, BL, NCHUNK, JL, N)
        # partition p = 8*il + bl; i = 16*chunk + il
        uc = uc.transpose(1, 2, 0, 3, 4).reshape(BL, I, JL, N)
        ufull[BL * bg : BL * bg + BL, :, JL * jg : JL * jg + JL, :] = uc
    sel = ufull[np.arange(B)[:, None], sidx]            # [B, NZC, J, N]
    return np.ascontiguousarray(sel.transpose(0, 2, 1, 3))  # [B, J, NZC, N]


def run_on_cores(input, W, trace=False, **trace_kwargs):
    from concourse.bass_utils import run_bass_kernel_spmd

    nc = _get_program()
    in_maps = _host_prep(input, W)
    res = run_bass_kernel_spmd(
        nc, in_maps, list(range(NCORES)), trace=trace, **trace_kwargs
    )
    return _host_finish(input, res.results), res


def kernel(input, W):
    out, _ = run_on_cores(input, W)
    return out



# revision 8
# speedup vs baseline: 2.8136x; 2.8136x over previous
"""Trainium2 Bass kernel for nn_DenseCapsuleLayer.

Reference computation:
    u_hat[b, j, k, n] = sum_m W[0, j, idx[b,k], n, m] * x[b, idx[b,k], m]
with idx[b, :] the ascending indices of the NZC=1152 non-zero child capsules
of batch b (x is zero elsewhere).

Strategy (8 NeuronCores, child-capsule sharding):
  * Core c owns i in [288*c, 288*(c+1)) -- 18 chunks of 16 child capsules --
    for ALL 32 parent capsules j and ALL 32 batches.  Each core computes the
    DENSE map u_full[b, i, j, n] over its i-range (x is zero at non-selected
    i, so those entries are zero and discarded); the select/compaction gather
    over i and the unshard happen on the host.
  * Everything on device is fp16 (inputs have small dynamic range; fp16
    keeps rel err ~1e-3 against the 2e-2 gate) with fp32 PSUM accumulation.
  * Per chunk g (16 i's) and batch-group bg (8 batches), the PE computes
        out[(il,bl), (j,n)] = sum_{il',m} bd[(il',m),(il,bl)] * W[(il',m),(j,n)]
    as ONE K=128, f=512 matmul filling a whole PSUM bank: the stationary is a
    [128,128] block-diagonal packing of x built ON DEVICE by one broadcast
    multiply per chunk (all 4 batch-groups at once, DVE 4x mode), the moving
    operand is the core's W chunk pre-transposed to [(il,m), (j,n)].
  * The 72 banks are drained (fp32->fp16) by DVE/ACT/Pool in region-sized
    runs so each region's output DMA carries a single-engine wait; input and
    output DMAs are spread over the SP/ACT/Pool queues, which the cost model
    runs concurrently.

Toolchain constraint: every lowered instruction accepts ONE sync-wait
command.  Dummy PE matmuls absorb extra dependencies (W-piece loads, the
block-diagonal RAW) so each real instruction keeps at most one wait; a BIR
post-pass collapses multiple waits on the SAME semaphore to the max tick.
"""

import numpy as np

B, I, J, M, N = 32, 2304, 32, 8, 16
NZC = I // 2
NCORES = 8
IL = I // NCORES          # 288 child capsules per core
NCHUNK = IL // 16         # 18 chunks of 16
NBG = 4                   # batch groups of 8
NBANK = NCHUNK * NBG      # 72 matmuls / PSUM-bank fills
JN = J * N                # 512 moving columns
REG = 6                   # banks per output region
NREG = NBANK // REG       # 12 output regions

# Drain-engine per region (d=DVE, a=ACT, p=Pool) and out-DMA queue per
# region (s=SP, a=ACT, p=Pool), tuned against the v1 cost model.
DRAIN_PAT = ["p", "d", "a", "p", "d", "a", "p", "d", "p", "d", "p", "a"]
OUTQ_PAT  = ["s", "p", "s", "a", "p", "s", "a", "p", "s", "a", "s", "p"]
# W load pieces: chunk ranges
W_PIECES = [(0, 3), (3, 8), (8, 13), (13, 18)]

_CACHE = {}


def _build_program():
    import concourse.bass as bass
    import concourse.mybir as mybir
    import concourse.tile as tile

    f16 = mybir.dt.float16
    f32 = mybir.dt.float32
    nc = bass.Bass()

    # wt[(il,m), g, (j,n)] = W[j, ibase + 16g + il, n, m]
    wt = nc.declare_dram_parameter("wt", [128, NCHUNK, JN], f16, isOutput=False)
    # xmk[:, 0:576]  = xc[(il,m), g, bg, bl] = x[8bg+bl, ibase+16g+il, m]
    # xmk[:, 576:704] = mask[(il,m), (il',bl)] = 1.0 iff il == il'
    xmk = nc.declare_dram_parameter("xmk", [128, NCHUNK * 32 + 128], f16,
                                    isOutput=False)
    u = nc.declare_dram_parameter("u", [128, NBANK, JN], f16, isOutput=True)

    with tile.TileContext(nc, pool_alloc_mode="queue") as tc:
        with (
            tc.tile_pool(name="wpool", bufs=1) as wpool,
            tc.tile_pool(name="opool", bufs=1) as opool,
            tc.tile_pool(name="ppool", bufs=7, space="PSUM") as ppool,
            tc.tile_pool(name="dpool", bufs=1, space="PSUM") as dpool,
            tc.tile_pool(name="zpool", bufs=1) as zpool,
        ):
            w_sb = wpool.tile([128, NCHUNK, JN], f16, tag="w")
            xm_sb = wpool.tile([128, NCHUNK * 32 + 128], f16, tag="xm")
            bdt = wpool.tile([128, NCHUNK, NBG, 16, 8], f16, tag="bd")
            o_sb = opool.tile([128, NBANK, JN], f16, tag="o")
            dmy = dpool.tile([1, 32], f32, tag="dmy")
            zz = zpool.tile([32, 2], f16, tag="zz")

            # PE pre-warm: start the p-state ramp before any DMA lands.
            nc.vector.memset(zz[:, :], 0.0)
            warm = nc.tensor.matmul(
                dmy[0:1, 31:32], zz[0:32, 0:1], zz[0:32, 0:1],
                start=True, stop=True,
            )

            # input DMAs: x+mask on ACT; W pieces on SP
            d_xm = nc.scalar.dma_start(out=xm_sb[:, :], in_=xmk[:, :])
            d_w = []
            for lo, hi in W_PIECES:
                d_w.append(
                    nc.sync.dma_start(out=w_sb[:, lo:hi, :], in_=wt[:, lo:hi, :])
                )

            xc5 = xm_sb[:, 0 : NCHUNK * 32].rearrange(
                "p (g bg one bl) -> p g bg one bl", g=NCHUNK, bg=NBG, one=1
            )
            mask5 = xm_sb[:, NCHUNK * 32 :].rearrange(
                "p (one r c) -> p one r c", one=1, r=16
            )

            piece_of = {}
            for pi, (lo, hi) in enumerate(W_PIECES):
                for g in range(lo, hi):
                    piece_of[g] = pi

            drains = {}       # bank k -> drain instruction
            drain_engine = {}
            ps_tiles = {}
            pending = []      # banks computed but not yet drained
            last_mm = None
            sdum_done = set()

            def engine_ns(e):
                return {"d": nc.vector, "a": nc.scalar, "p": nc.gpsimd}[e]

            def emit_drains(upto_bank):
                while pending and pending[0] <= upto_bank:
                    k = pending.pop(0)
                    e = DRAIN_PAT[k // REG]
                    if e == "a":
                        cp = nc.scalar.copy(o_sb[:, k, :], ps_tiles[k][:, :])
                    else:
                        cp = engine_ns(e).tensor_copy(
                            o_sb[:, k, :], ps_tiles[k][:, :]
                        )
                    drains[k] = cp
                    drain_engine[k] = e

            for g in range(NCHUNK):
                # one block-diagonal build for all 4 batch groups of chunk g
                mul = nc.vector.tensor_mul(
                    bdt[:, g, :, :, :],
                    xc5[:, g].broadcast_to([128, NBG, 16, 8]),
                    mask5[:, :].broadcast_to([128, NBG, 16, 8]),
                )
                # PE dummy advancing the PE clock past mul[g] (and, at W piece
                # boundaries, a second dummy past the piece's DMA)
                gdum = nc.tensor.matmul(
                    dmy[0:1, g : g + 1],
                    bdt[0:32, g, 0, 0, 0:1],
                    bdt[0:32, g, 0, 0, 0:1],
                    start=True, stop=True,
                )
                pi = piece_of[g]
                if pi not in sdum_done:
                    sdum_done.add(pi)
                    sdum = nc.tensor.matmul(
                        dmy[0:1, 20 + pi : 21 + pi],
                        w_sb[0:32, g, 0:1],
                        w_sb[0:32, g, 0:1],
                        start=True, stop=True,
                    )
                    tile.add_dep_helper(
                        sdum.ins, gdum.ins, sync=False, reason="order dummies"
                    )
                for bg in range(NBG):
                    k = NBG * g + bg
                    ps = ppool.tile([128, JN], f32, tag="ps")
                    ps_tiles[k] = ps
                    mm = nc.tensor.matmul(
                        ps[:, :],
                        bdt[:, g, bg, :, :],
                        w_sb[:, g, :],
                        start=True, stop=True,
                    )
                    tile.add_dep_helper(
                        mm.ins, gdum.ins, sync=False, reason="gdum before mm"
                    )
                    last_mm = mm
                    pending.append(k)
                # drain banks a little behind the matmul front
                emit_drains(NBG * g + NBG - 1 - 6)
            emit_drains(NBANK - 1)

            # output DMAs, one per region, queue per OUTQ_PAT
            d_out = []
            for r in range(NREG):
                q = {"s": nc.sync, "a": nc.scalar, "p": nc.gpsimd}[OUTQ_PAT[r]]
                d_out.append(
                    q.dma_start(
                        out=u[:, r * REG : (r + 1) * REG, :],
                        in_=o_sb[:, r * REG : (r + 1) * REG, :],
                    )
                )

            # tail parking: chain SP nops so the kernel-tail drain has at
            # most one wait left per instruction
            def park_wait(dep, prev=None):
                w = nc.sync.nop(nofuse=True, hint="park")
                tile.add_dep_helper(w.ins, dep.ins, sync=True, reason="park")
                if prev is not None:
                    tile.add_dep_helper(w.ins, prev.ins, sync=False, reason="ord")
                return w

            last_drain_of = {}
            for k in range(NBANK):
                last_drain_of[DRAIN_PAT[k // REG]] = drains[k]
            prev = None
            for d in (
                [d_xm] + d_w + d_out + [last_mm, warm]
                + list(last_drain_of.values())
            ):
                prev = park_wait(d, prev)

    # Single-wait legalization: collapse multiple waits on the same semaphore
    # to the single max-tick wait; anything still multi-sem is a bug.
    import concourse.mybir as mybir2

    for blk in nc.m.functions[0].blocks:
        for inst in blk.instructions:
            si = inst.sync_info
            if si is None or not si.on_wait or len(si.on_wait) < 2:
                continue
            by_sem = {}
            for w in si.on_wait:
                prev_w = by_sem.get(w.ant_name)
                if prev_w is None or w.wait_value > prev_w.wait_value:
                    by_sem[w.ant_name] = w
            keep = list(by_sem.values())
            if len(keep) > 1 and type(inst).__name__ == "InstDMACopy":
                # same-queue DMAHW ordering waits are implied transitively
                # (this DMA's engine wait covers every input DMA through the
                # mul/dummy/matmul/drain chain); the queue also executes its
                # DMAs in order regardless.
                non_dma = [w for w in keep if not w.ant_name.startswith("DMAHW")]
                if len(non_dma) == 1:
                    keep = non_dma
            if len(keep) > 1:
                raise RuntimeError(
                    f"multi-sem wait on {inst.name} ({type(inst).__name__}): "
                    f"{[(w.ant_name, w.wait_value) for w in keep]}"
                )
            inst.sync_info = mybir2.SyncInfo(
                on_wait=keep, on_update=list(si.on_update or [])
            )
    return nc


def _get_program():
    if "nc" not in _CACHE:
        _CACHE["nc"] = _build_program()
    return _CACHE["nc"]


def _host_prep(input, W):
    """Build per-core in_maps. input: [B, I, M]; W: [1, J, I, N, M]."""
    x = np.asarray(input, dtype=np.float32)
    W0 = np.asarray(W, dtype=np.float32)[0]             # [J, I, N, M]

    # mask[(il, m), (il', bl)] = 1 iff il == il'
    il_row = (np.arange(128) // M)[:, None]
    il_col = (np.arange(128) // 8)[None, :]
    mask = (il_row == il_col).astype(np.float16)        # [128, 128]

    # wt_all[(il,m), i-chunk-global, (j,n)]
    # row p = 8*il + m ; value W0[j, i, n, m] at col 16*j + n
    wt_all = W0.transpose(1, 3, 0, 2).reshape(I, M, JN) # [i, m, (j,n)]
    wt_all = wt_all.reshape(NCORES, NCHUNK, 16, M, JN).astype(np.float16)

    # xc_all[b, i, m] -> per core [128=(il,m), g, bg, bl]
    xs = x.reshape(B, NCORES, NCHUNK, 16, M).astype(np.float16)

    in_maps = []
    for c in range(NCORES):
        wt_c = wt_all[c].transpose(1, 2, 0, 3).reshape(128, NCHUNK, JN)
        # xc[(il,m), g, bg, bl] = x[8bg+bl, ...]
        xc = xs[:, c].transpose(2, 3, 1, 0)             # [16, M, g, B]
        xc = xc.reshape(128, NCHUNK, NBG, 8)
        xmk = np.concatenate(
            [xc.reshape(128, NCHUNK * 32), mask.reshape(128, 128)], axis=1
        )
        in_maps.append(
            {"wt": np.ascontiguousarray(wt_c), "xmk": np.ascontiguousarray(xmk)}
        )
    return in_maps


def _host_finish(input, results):
    """Gather selected child capsules and unshard over i."""
    mask = input.sum(axis=2) != 0.0                     # [B, I]
    keyv = np.where(mask, np.arange(I)[None, :], I)
    sidx = np.sort(keyv, axis=1)[:, :NZC]               # [B, NZC]

    ufull = np.empty((B, I, J, N), dtype=np.float32)
    for c in range(NCORES):
        uc = results[c]["u"].astype(np.float32)         # [128, NBANK, JN]
        # p = 8*il + bl ; bank k = 4g + bg ; col f = 16*j + n
        uc = uc.reshape(16, 8, NCHUNK, NBG, J, N)       # [il, bl, g, bg, j, n]
        # -> [b, i_local, j, n] with b = 8*bg + bl, i_local = 16*g + il
        uc = uc.transpose(3, 1, 2, 0, 4, 5).reshape(B, IL, J, N)
        ufull[:, IL * c : IL * (c + 1)] = uc
    sel = ufull[np.arange(B)[:, None], sidx]            # [B, NZC, J, N]
    return np.ascontiguousarray(sel.transpose(0, 2, 1, 3))  # [B, J, NZC, N]


def run_on_cores(input, W, trace=False, **trace_kwargs):
    from concourse.bass_utils import run_bass_kernel_spmd

    nc = _get_program()
    in_maps = _host_prep(input, W)
    res = run_bass_kernel_spmd(
        nc, in_maps, list(range(NCORES)), trace=trace, **trace_kwargs
    )
    return _host_finish(input, res.results), res


def kernel(input, W):
    out, _ = run_on_cores(input, W)
    return out
